# revision 9
# baseline (speedup 1.0000x reference)
"""Trainium2 Bass kernel for vq_codebook (EucCluster) problem.

Computation (reference):
    dists = ||embs[r] - centers[k]||           (N=200000, K=256, D=128)
    cluster_dists = min_k dists                -> loss = sum_r cluster_dists
    rep_ids = argmin_r dists (per center)      -> centers_out = embs[rep_ids]

Strategy:
  - Shard embs row-wise across 8 cores (25088 rows/core after padding to
    200704 with a far-away constant). Host pre-transposes embs so each core
    receives embsT [128=dim partitions, 25088 rows] (contiguous DMA lines).
  - Per 256-row block, on each core:
      PE (fp32r matmuls @ 1 cyc/row since N=256):
        e2row[1,256]   = ones^T @ (E^2)                 (row squared-norms)
        e2col[128,1]x2 = (E^2)_sub @ ones               (column form, PSUM)
        psB[k,r] x2    = (2C^T)_h^T @ E  - e2row        (= 2ec - e2, rank-1 acc)
        psA[r,k] x2    = E_sub^T @ (-2C^T)              (= -2ec)
      ACT: E^2 square, e2row PSUM->SBUF copy, psB -> SBUF f32 copy (D2)
      DVE:
        tensor_tensor_reduce: min_k(psA + c2_bcast) -> Amin psum column
          (e2[r] is constant along k, added after the min via e2col)
        max/max_index on D2 (= 2ec - e2 = -(d2 - c2[k])): per-block top-8
          candidate values + indices per center  (c2[k] const per center)
  - Epilogue: Amin + E2col -> sqrt -> mask pad rows -> row sum -> partition
    sum via matmul -> loss partial per core.
  - Host: merges per-block top-8 candidates per center, rescores the best
    few in f64 to pick exact argmin (immune to fp32r rounding), gathers
    centers_out, sums loss partials.
"""

import sys

import numpy as np

sys.path.insert(0, "/opt/trn_rl_repo")

N = 200000
D = 128
K = 256
NCORES = 8
BLK = 256  # rows per block
NBLK = 98  # blocks per core
RCORE = NBLK * BLK  # 25088
NPAD = RCORE * NCORES  # 200704
NSUB = 2 * NBLK  # 196 subtiles (128 rows each) per core
PADVAL = 1.0e4
RESCORE_TOP = 24  # candidates rescored exactly per center on host

_CACHE = {}


def build_program(nblk=NBLK):
    import concourse.bacc as bacc
    import concourse.mybir as mybir
    from concourse import tile

    dt = mybir.dt
    f32 = dt.float32
    f32r = dt.float32r
    Act = mybir.ActivationFunctionType
    Alu = mybir.AluOpType
    X = mybir.AxisListType.X

    rcore = nblk * BLK
    nsub = 2 * nblk

    nc = bacc.Bacc("TRN2", target_bir_lowering=False, debug=False, num_devices=NCORES)

    embsT = nc.dram_tensor("embs_t", [D, rcore], f32r, kind="ExternalInput")
    ct2 = nc.dram_tensor("ct2", [D, K], f32r, kind="ExternalInput")  # +2*C^T
    ctn2 = nc.dram_tensor("ctn2", [D, K], f32r, kind="ExternalInput")  # -2*C^T
    maskd = nc.dram_tensor("mask", [128, nsub], f32, kind="ExternalInput")
    onescol_d = nc.dram_tensor("ones_col_in", [128, 2], f32r, kind="ExternalInput")
    nones_d = nc.dram_tensor("nones_in", [2, 128], f32r, kind="ExternalInput")
    onesrow_d = nc.dram_tensor("onesrow_in", [2, 128], f32r, kind="ExternalInput")
    cval_d = nc.dram_tensor("cand_val", [2, 128, nblk * 8], f32, kind="ExternalOutput")
    cidx_d = nc.dram_tensor("cand_idx", [2, 128, nblk * 8], dt.uint16, kind="ExternalOutput")
    loss_d = nc.dram_tensor("loss_part", [128, 1], f32, kind="ExternalOutput")

    with tile.TileContext(nc) as tc:
        with (
            tc.tile_pool(name="const", bufs=1) as constp,
            tc.tile_pool(name="io", bufs=4) as iop,
            tc.tile_pool(name="work", bufs=3) as workp,
            tc.tile_pool(name="psmall", bufs=2, space="PSUM") as psmallp,
            tc.tile_pool(name="pbig", bufs=2, space="PSUM") as pbigp,
            tc.tile_pool(name="pacc", bufs=1, space="PSUM") as paccp,
        ):
            # ---- constants ----
            ct2_sb = constp.tile([D, K], f32r)
            nc.sync.dma_start(ct2_sb[:], ct2[:, :])
            ctn2_sb = constp.tile([D, K], f32r)
            nc.sync.dma_start(ctn2_sb[:], ctn2[:, :])
            mask_sb = constp.tile([128, nsub], f32)
            nc.sync.dma_start(mask_sb[:], maskd[:, :])
            ones_col2 = constp.tile([128, 2], f32r)
            nc.sync.dma_start(ones_col2[:], onescol_d[:, :])
            nones2_t = constp.tile([2, 128], f32r)
            nc.sync.dma_start(nones2_t[:], nones_d[:, :])
            ones2_t = constp.tile([2, 128], f32r)
            nc.sync.dma_start(ones2_t[:], onesrow_d[:, :])
            nones2 = nones2_t[:]
            ones2 = ones2_t[:]

            # c2row[1,K] = sum_d C^2 = sum_d (2C^T)^2 / 4
            csq = workp.tile([D, K], f32r, tag="csq")
            nc.scalar.activation(csq[:], ct2_sb[:], Act.Square)
            c2_ps = psmallp.tile([2, K], f32, tag="e2row")
            nc.tensor.matmul(c2_ps[:], ones_col2[:], csq[:])
            # both rows = 4*c2; scale to c2/2 so a K=2 ones-matmul adds c2
            c2pair = constp.tile([2, K], f32r)
            nc.scalar.activation(c2pair[:], c2_ps[:], Act.Copy, scale=0.125)

            # ---- persistent accumulators ----
            amin_ps = paccp.tile([128, nsub], f32, tag="amin")  # min_k(-2ec + c2)
            # e2 column form, written as even/odd duplicate pairs (fp32r matmul
            # requires even output free size)
            e2col_ps = paccp.tile([128, 2 * nsub], f32, tag="e2col")
            cval_sb = [
                constp.tile([128, nblk * 8], f32, name=f"cval{h}", tag=f"cval{h}")
                for h in range(2)
            ]
            cidx_sb = [
                constp.tile([128, nblk * 8], dt.uint16, name=f"cidx{h}", tag=f"cidx{h}")
                for h in range(2)
            ]

            # ---- main loop over row blocks ----
            for t in range(nblk):
                et = iop.tile([D, BLK], f32r, tag="et")
                nc.sync.dma_start(et[:], embsT[:, t * BLK : (t + 1) * BLK])

                esq = workp.tile([D, BLK], f32r, tag="esq")
                nc.scalar.activation(esq[:], et[:], Act.Square)

                e2_ps = psmallp.tile([2, BLK], f32, tag="e2row")
                nc.tensor.matmul(e2_ps[:], ones_col2[:], esq[:])
                # both rows = e2; scale to e2/2 so a K=2 (-1)s-matmul subtracts e2
                e2pair = workp.tile([2, BLK], f32r, tag="e2pair")
                nc.scalar.activation(e2pair[:], e2_ps[:], Act.Copy, scale=0.5)

                # e2 in column form -> persistent PSUM (added to Amin at end)
                for s in range(2):
                    col = 2 * t + s
                    nc.tensor.matmul(
                        e2col_ps[:, 2 * col : 2 * col + 2],
                        esq[:, s * 128 : (s + 1) * 128],
                        ones_col2[:],
                    )

                # ---- B side: psB[k, r] = 2ec - e2 ----
                for h in range(2):
                    psb = pbigp.tile([128, BLK], f32, tag="psb")
                    nc.tensor.matmul(
                        psb[:],
                        ct2_sb[:, h * 128 : (h + 1) * 128],
                        et[:],
                        start=True,
                        stop=False,
                    )
                    nc.tensor.matmul(
                        psb[:],
                        nones2,
                        e2pair[:],
                        start=False,
                        stop=True,
                    )
                    d2 = workp.tile([128, BLK], f32, tag="d2")
                    nc.scalar.activation(d2[:], psb[:], Act.Copy)
                    vsl = cval_sb[h][:, t * 8 : (t + 1) * 8]
                    nc.vector.max(vsl, d2[:])
                    nc.vector.max_index(cidx_sb[h][:, t * 8 : (t + 1) * 8], vsl, d2[:])

                # ---- A side: amin[r] = min_k(-2ec + c2) ----
                for s in range(2):
                    psa = pbigp.tile([128, K], f32, tag="psa")
                    nc.tensor.matmul(
                        psa[:],
                        et[:, s * 128 : (s + 1) * 128],
                        ctn2_sb[:],
                        start=True,
                        stop=False,
                    )
                    nc.tensor.matmul(
                        psa[:],
                        ones2,
                        c2pair[:],
                        start=False,
                        stop=True,
                    )
                    col = 2 * t + s
                    nc.vector.tensor_reduce(
                        amin_ps[:, col : col + 1], psa[:], axis=X, op=Alu.min
                    )

            # ---- epilogue: loss ----
            e2col_sb = workp.tile([128, nsub], f32, tag="e2colsb")
            e2col_even = e2col_ps[:].rearrange("p (n two) -> p n two", two=2)[:, :, 0]
            nc.scalar.activation(e2col_sb[:], e2col_even, Act.Copy)
            dmin = workp.tile([128, nsub], f32, tag="dmin")
            nc.vector.tensor_tensor(dmin[:], amin_ps[:], e2col_sb[:], op=Alu.add)
            nc.vector.tensor_scalar_max(dmin[:], dmin[:], 1e-12)
            dsq = workp.tile([128, nsub], f32, tag="dsq")
            nc.scalar.activation(dsq[:], dmin[:], Act.Sqrt)
            dm = workp.tile([128, nsub], f32, tag="dm")
            nc.vector.tensor_tensor(dm[:], dsq[:], mask_sb[:], op=Alu.mult)
            rsum = workp.tile([128, 1], f32, tag="rsum")
            nc.vector.tensor_reduce(rsum[:], dm[:], axis=X, op=Alu.add)
            nc.sync.dma_start(loss_d[:, :], rsum[:])
            for h in range(2):
                nc.sync.dma_start(cval_d[h], cval_sb[h][:])
                nc.sync.dma_start(cidx_d[h], cidx_sb[h][:])

    nc.compile()
    return nc


def _get_program(nblk=NBLK):
    if nblk not in _CACHE:
        _CACHE[nblk] = build_program(nblk)
    return _CACHE[nblk]


def make_in_maps(embs, centers, nblk=NBLK, ncores=NCORES):
    """Host-side shard prep: pad, transpose, per-core input dicts."""
    rcore = nblk * BLK
    npad = rcore * ncores
    nsub = 2 * nblk
    n = embs.shape[0]

    embs_pad = np.full((npad, D), PADVAL, dtype=np.float32)
    embs_pad[:n] = embs
    embs_t = np.ascontiguousarray(embs_pad.T)  # [D, npad]

    ct2 = np.ascontiguousarray((2.0 * centers.T).astype(np.float32))
    ctn2 = np.ascontiguousarray((-2.0 * centers.T).astype(np.float32))


    in_maps = []
    for c in range(ncores):
        base = c * rcore
        shard = np.ascontiguousarray(embs_t[:, base : base + rcore])
        # mask[p, 2t+s] = 1 if global row base + t*256 + s*128 + p is real
        cols = np.arange(nsub)
        rows = base + (cols // 2) * BLK + (cols % 2) * 128
        gl = rows[None, :] + np.arange(128)[:, None]
        mask = (gl < n).astype(np.float32)
        in_maps.append(
            {
                "embs_t": shard,
                "ct2": ct2,
                "ctn2": ctn2,
                "mask": np.ascontiguousarray(mask),
                "ones_col_in": np.ones((128, 2), dtype=np.float32),
                "nones_in": np.full((2, 128), -1.0, dtype=np.float32),
                "onesrow_in": np.ones((2, 128), dtype=np.float32),
            }
        )
    return in_maps


def postprocess(results, embs, centers, nblk=NBLK, ncores=NCORES):
    """Merge per-core device outputs into (centers_out, rep_ids, loss)."""
    rcore = nblk * BLK
    n = embs.shape[0]

    # Candidate global rows + values per center.
    all_vals = []  # [K, ncands] value = 2ec - e2 (bigger = smaller d2)
    all_rows = []
    loss = 0.0
    for c, res in enumerate(results):
        base = c * rcore
        cv = np.asarray(res["cand_val"])  # [2, 128, nblk*8]
        ci = np.asarray(res["cand_idx"]).astype(np.int64)  # [2, 128, nblk*8]
        loss += float(np.asarray(res["loss_part"], dtype=np.float64).sum())
        blk_of = np.repeat(np.arange(nblk), 8)  # candidate slot -> block
        rows = base + blk_of[None, None, :] * BLK + ci  # [2, 128, nblk*8]
        all_vals.append(cv.reshape(K, -1))
        all_rows.append(rows.reshape(K, -1))
    vals = np.concatenate(all_vals, axis=1)  # [K, total]
    rows = np.concatenate(all_rows, axis=1)
    vals = np.where(rows < n, vals, -np.inf)

    top = min(RESCORE_TOP, vals.shape[1])
    part = np.argpartition(-vals, top - 1, axis=1)[:, :top]  # [K, top]
    cand_rows = np.take_along_axis(rows, part, axis=1)  # [K, top]
    cand_rows = np.clip(cand_rows, 0, n - 1)

    # Exact rescore in f64.
    e = embs[cand_rows].astype(np.float64)  # [K, top, D]
    cc = centers.astype(np.float64)[:, None, :]  # [K, 1, D]
    d2 = np.sum((e - cc) ** 2, axis=2)  # [K, top]
    cand_vals_valid = np.take_along_axis(vals, part, axis=1) > -np.inf
    d2 = np.where(cand_vals_valid, d2, np.inf)
    # argmin with ties broken by lowest global row (matches jnp.argmin order)
    order = np.lexsort((cand_rows, d2), axis=1)[:, 0]
    rep_ids = np.take_along_axis(cand_rows, order[:, None], axis=1)[:, 0]
    rep_ids = rep_ids.astype(np.int32)

    centers_out = embs[rep_ids].astype(np.float32)
    return centers_out, rep_ids, np.float32(loss)


def kernel(embs, centers):
    from concourse.bass_utils import run_bass_kernel_spmd

    embs = np.asarray(embs, dtype=np.float32)
    centers = np.asarray(centers, dtype=np.float32)

    nc = _get_program()
    in_maps = make_in_maps(embs, centers)
    res = run_bass_kernel_spmd(nc, in_maps, core_ids=list(range(NCORES)))
    return postprocess(res.results, embs, centers)


# revision 11
# speedup vs baseline: 2.5320x; 2.5320x over previous
"""Trainium2 Bass kernel for vq_codebook (EucCluster) problem.

Computation (reference):
    dists = ||embs[r] - centers[k]||           (N=200000, K=256, D=128)
    cluster_dists = min_k dists                -> loss = sum_r cluster_dists
    rep_ids = argmin_r dists (per center)      -> centers_out = embs[rep_ids]

Strategy:
  - Shard embs row-wise across 8 cores (25088 rows/core after padding to
    200704 with a far-away constant). Host pre-transposes embs so each core
    receives embsT [128=dim partitions, 25088 rows] (contiguous DMA lines).
  - Per 256-row block, on each core:
      PE (fp32r matmuls @ 1 cyc/row since N=256):
        e2row[1,256]   = ones^T @ (E^2)                 (row squared-norms)
        e2col[128,1]x2 = (E^2)_sub @ ones               (column form, PSUM)
        psB[k,r] x2    = (2C^T)_h^T @ E  - e2row        (= 2ec - e2, K=2 acc)
        psA[r,k] x2    = E_sub^T @ (-2C^T) + c2         (K=2 acc)
      ACT: E^2 square, e2 pair PSUM->SBUF copy, psB -> SBUF f32 copy (D2)
      DVE:
        tensor_reduce min_k(psA) -> Amin psum column (c2 folded into psA by a
          K=2 ones-matmul; e2[r] is const along k, added after the min)
        max/max_index on D2 (= 2ec - e2 = -(d2 - c2[k])): per-block top-8
          candidate values + indices per center  (c2[k] const per center)
  - Epilogue: Amin + E2col -> sqrt -> mask pad rows -> row sum -> partition
    sum via matmul -> loss partial per core.
  - Host: merges per-block top-8 candidates per center, rescores the best
    few in f64 to pick exact argmin (immune to fp32r rounding), gathers
    centers_out, sums loss partials.
"""

import sys

import numpy as np

sys.path.insert(0, "/opt/trn_rl_repo")

N = 200000
D = 128
K = 256
NCORES = 8
BLK = 256  # rows per block
NBLK = 98  # blocks per core
RCORE = NBLK * BLK  # 25088
NPAD = RCORE * NCORES  # 200704
NSUB = 2 * NBLK  # 196 subtiles (128 rows each) per core
PADVAL = 1.0e4
RESCORE_TOP = 24  # candidates rescored exactly per center on host

_CACHE = {}


def build_program(nblk=NBLK):
    import concourse.bacc as bacc
    import concourse.mybir as mybir
    from concourse import tile

    dt = mybir.dt
    f32 = dt.float32
    f32r = dt.float32r
    Act = mybir.ActivationFunctionType
    Alu = mybir.AluOpType
    X = mybir.AxisListType.X

    rcore = nblk * BLK
    nsub = 2 * nblk

    nc = bacc.Bacc("TRN2", target_bir_lowering=False, debug=False, num_devices=NCORES)

    embsT = nc.dram_tensor("embs_t", [D, rcore], f32r, kind="ExternalInput")
    ct2 = nc.dram_tensor("ct2", [D, K], f32r, kind="ExternalInput")  # +2*C^T
    ctn2 = nc.dram_tensor("ctn2", [D, K], f32r, kind="ExternalInput")  # -2*C^T
    maskd = nc.dram_tensor("mask", [128, nsub], f32, kind="ExternalInput")
    onescol_d = nc.dram_tensor("ones_col_in", [128, 2], f32r, kind="ExternalInput")
    nones_d = nc.dram_tensor("nones_in", [2, 128], f32r, kind="ExternalInput")
    onesrow_d = nc.dram_tensor("onesrow_in", [2, 128], f32r, kind="ExternalInput")
    cval_d = nc.dram_tensor("cand_val", [2, 128, nblk * 8], f32, kind="ExternalOutput")
    cidx_d = nc.dram_tensor("cand_idx", [2, 128, nblk * 8], dt.uint16, kind="ExternalOutput")
    loss_d = nc.dram_tensor("loss_part", [128, 1], f32, kind="ExternalOutput")

    with tile.TileContext(nc) as tc:
        with (
            tc.tile_pool(name="const", bufs=1) as constp,
            tc.tile_pool(name="io", bufs=4) as iop,
            tc.tile_pool(name="work", bufs=3) as workp,
            tc.tile_pool(name="psmall", bufs=2, space="PSUM") as psmallp,
            tc.tile_pool(name="pbig", bufs=2, space="PSUM") as pbigp,
            tc.tile_pool(name="pacc", bufs=1, space="PSUM") as paccp,
        ):
            # ---- constants ----
            ct2_sb = constp.tile([D, K], f32r)
            nc.sync.dma_start(ct2_sb[:], ct2[:, :])
            ctn2_sb = constp.tile([D, K], f32r)
            nc.sync.dma_start(ctn2_sb[:], ctn2[:, :])
            mask_sb = constp.tile([128, nsub], f32)
            nc.sync.dma_start(mask_sb[:], maskd[:, :])
            ones_col2 = constp.tile([128, 2], f32r)
            nc.sync.dma_start(ones_col2[:], onescol_d[:, :])
            nones2_t = constp.tile([2, 128], f32r)
            nc.sync.dma_start(nones2_t[:], nones_d[:, :])
            ones2_t = constp.tile([2, 128], f32r)
            nc.sync.dma_start(ones2_t[:], onesrow_d[:, :])
            nones2 = nones2_t[:]
            ones2 = ones2_t[:]

            # c2row[1,K] = sum_d C^2 = sum_d (2C^T)^2 / 4
            csq = workp.tile([D, K], f32r, tag="csq")
            nc.scalar.activation(csq[:], ct2_sb[:], Act.Square)
            c2_ps = psmallp.tile([2, K], f32, tag="e2row")
            nc.tensor.matmul(c2_ps[:], ones_col2[:], csq[:])
            # both rows = 4*c2; scale to c2/2 so a K=2 ones-matmul adds c2
            c2pair = constp.tile([2, K], f32r)
            nc.scalar.activation(c2pair[:], c2_ps[:], Act.Copy, scale=0.125)

            # ---- persistent accumulators ----
            amin_ps = paccp.tile([128, nsub], f32, tag="amin")  # min_k(-2ec + c2)
            # e2 column form, written as even/odd duplicate pairs (fp32r matmul
            # requires even output free size)
            e2col_ps = paccp.tile([128, 2 * nsub], f32, tag="e2col")
            cval_sb = [
                constp.tile([128, nblk * 8], f32, name=f"cval{h}", tag=f"cval{h}")
                for h in range(2)
            ]
            cidx_sb = [
                constp.tile([128, nblk * 8], dt.uint16, name=f"cidx{h}", tag=f"cidx{h}")
                for h in range(2)
            ]

            # ---- main loop over row blocks (DMA 4 blocks at a time) ----
            DMAB = 4
            et4 = None
            for t in range(nblk):
                if t % DMAB == 0:
                    nload = min(DMAB, nblk - t) * BLK
                    et4 = iop.tile([D, DMAB * BLK], f32r, tag="et4")
                    nc.sync.dma_start(
                        et4[:, :nload], embsT[:, t * BLK : t * BLK + nload]
                    )
                et = et4[:, (t % DMAB) * BLK : (t % DMAB + 1) * BLK]

                esq = workp.tile([D, BLK], f32r, tag="esq")
                nc.scalar.activation(esq[:], et, Act.Square)

                e2_ps = psmallp.tile([2, BLK], f32, tag="e2row")
                nc.tensor.matmul(e2_ps[:], ones_col2[:], esq[:])
                # both rows = e2; scale to e2/2 so a K=2 (-1)s-matmul subtracts e2
                e2pair = workp.tile([2, BLK], f32r, tag="e2pair")
                nc.scalar.activation(e2pair[:], e2_ps[:], Act.Copy, scale=0.5)

                # e2 in column form -> persistent PSUM (added to Amin at end)
                for s in range(2):
                    col = 2 * t + s
                    nc.tensor.matmul(
                        e2col_ps[:, 2 * col : 2 * col + 2],
                        esq[:, s * 128 : (s + 1) * 128],
                        ones_col2[:],
                    )

                # ---- B side: psB[k, r] = 2ec - e2 ----
                for h in range(2):
                    psb = pbigp.tile([128, BLK], f32, tag="psb")
                    nc.tensor.matmul(
                        psb[:],
                        ct2_sb[:, h * 128 : (h + 1) * 128],
                        et,
                        start=True,
                        stop=False,
                    )
                    nc.tensor.matmul(
                        psb[:],
                        nones2,
                        e2pair[:],
                        start=False,
                        stop=True,
                    )
                    d2 = workp.tile([128, BLK], f32, tag="d2")
                    nc.scalar.activation(d2[:], psb[:], Act.Copy)
                    vsl = cval_sb[h][:, t * 8 : (t + 1) * 8]
                    nc.vector.max(vsl, d2[:])
                    nc.vector.max_index(cidx_sb[h][:, t * 8 : (t + 1) * 8], vsl, d2[:])

                # ---- A side: amin[r] = min_k(-2ec + c2) ----
                for s in range(2):
                    psa = pbigp.tile([128, K], f32, tag="psa")
                    nc.tensor.matmul(
                        psa[:],
                        et4[:, (t % DMAB) * BLK + s * 128 : (t % DMAB) * BLK + (s + 1) * 128],
                        ctn2_sb[:],
                        start=True,
                        stop=False,
                    )
                    nc.tensor.matmul(
                        psa[:],
                        ones2,
                        c2pair[:],
                        start=False,
                        stop=True,
                    )
                    col = 2 * t + s
                    nc.vector.tensor_reduce(
                        amin_ps[:, col : col + 1], psa[:], axis=X, op=Alu.min
                    )

            # ---- epilogue: loss ----
            e2col_sb = workp.tile([128, nsub], f32, tag="e2colsb")
            e2col_even = e2col_ps[:].rearrange("p (n two) -> p n two", two=2)[:, :, 0]
            nc.scalar.activation(e2col_sb[:], e2col_even, Act.Copy)
            dmin = workp.tile([128, nsub], f32, tag="dmin")
            nc.vector.tensor_tensor(dmin[:], amin_ps[:], e2col_sb[:], op=Alu.add)
            nc.vector.tensor_scalar_max(dmin[:], dmin[:], 1e-12)
            dsq = workp.tile([128, nsub], f32, tag="dsq")
            nc.scalar.activation(dsq[:], dmin[:], Act.Sqrt)
            dm = workp.tile([128, nsub], f32, tag="dm")
            nc.vector.tensor_tensor(dm[:], dsq[:], mask_sb[:], op=Alu.mult)
            rsum = workp.tile([128, 1], f32, tag="rsum")
            nc.vector.tensor_reduce(rsum[:], dm[:], axis=X, op=Alu.add)
            nc.sync.dma_start(loss_d[:, :], rsum[:])
            for h in range(2):
                nc.sync.dma_start(cval_d[h], cval_sb[h][:])
                nc.sync.dma_start(cidx_d[h], cidx_sb[h][:])

    nc.compile()
    return nc


def _get_program(nblk=NBLK):
    if nblk not in _CACHE:
        _CACHE[nblk] = build_program(nblk)
    return _CACHE[nblk]


def make_in_maps(embs, centers, nblk=NBLK, ncores=NCORES):
    """Host-side shard prep: pad, transpose, per-core input dicts."""
    rcore = nblk * BLK
    npad = rcore * ncores
    nsub = 2 * nblk
    n = embs.shape[0]

    embs_pad = np.full((npad, D), PADVAL, dtype=np.float32)
    embs_pad[:n] = embs
    embs_t = np.ascontiguousarray(embs_pad.T)  # [D, npad]

    ct2 = np.ascontiguousarray((2.0 * centers.T).astype(np.float32))
    ctn2 = np.ascontiguousarray((-2.0 * centers.T).astype(np.float32))


    in_maps = []
    for c in range(ncores):
        base = c * rcore
        shard = np.ascontiguousarray(embs_t[:, base : base + rcore])
        # mask[p, 2t+s] = 1 if global row base + t*256 + s*128 + p is real
        cols = np.arange(nsub)
        rows = base + (cols // 2) * BLK + (cols % 2) * 128
        gl = rows[None, :] + np.arange(128)[:, None]
        mask = (gl < n).astype(np.float32)
        in_maps.append(
            {
                "embs_t": shard,
                "ct2": ct2,
                "ctn2": ctn2,
                "mask": np.ascontiguousarray(mask),
                "ones_col_in": np.ones((128, 2), dtype=np.float32),
                "nones_in": np.full((2, 128), -1.0, dtype=np.float32),
                "onesrow_in": np.ones((2, 128), dtype=np.float32),
            }
        )
    return in_maps


def postprocess(results, embs, centers, nblk=NBLK, ncores=NCORES):
    """Merge per-core device outputs into (centers_out, rep_ids, loss)."""
    rcore = nblk * BLK
    n = embs.shape[0]

    # Candidate global rows + values per center.
    all_vals = []  # [K, ncands] value = 2ec - e2 (bigger = smaller d2)
    all_rows = []
    loss = 0.0
    for c, res in enumerate(results):
        base = c * rcore
        cv = np.asarray(res["cand_val"])  # [2, 128, nblk*8]
        ci = np.asarray(res["cand_idx"]).astype(np.int64)  # [2, 128, nblk*8]
        loss += float(np.asarray(res["loss_part"], dtype=np.float64).sum())
        blk_of = np.repeat(np.arange(nblk), 8)  # candidate slot -> block
        rows = base + blk_of[None, None, :] * BLK + ci  # [2, 128, nblk*8]
        all_vals.append(cv.reshape(K, -1))
        all_rows.append(rows.reshape(K, -1))
    vals = np.concatenate(all_vals, axis=1)  # [K, total]
    rows = np.concatenate(all_rows, axis=1)
    vals = np.where(rows < n, vals, -np.inf)

    top = min(RESCORE_TOP, vals.shape[1])
    part = np.argpartition(-vals, top - 1, axis=1)[:, :top]  # [K, top]
    cand_rows = np.take_along_axis(rows, part, axis=1)  # [K, top]
    cand_rows = np.clip(cand_rows, 0, n - 1)

    # Exact rescore in f64.
    e = embs[cand_rows].astype(np.float64)  # [K, top, D]
    cc = centers.astype(np.float64)[:, None, :]  # [K, 1, D]
    d2 = np.sum((e - cc) ** 2, axis=2)  # [K, top]
    cand_vals_valid = np.take_along_axis(vals, part, axis=1) > -np.inf
    d2 = np.where(cand_vals_valid, d2, np.inf)
    # argmin with ties broken by lowest global row (matches jnp.argmin order)
    order = np.lexsort((cand_rows, d2), axis=1)[:, 0]
    rep_ids = np.take_along_axis(cand_rows, order[:, None], axis=1)[:, 0]
    rep_ids = rep_ids.astype(np.int32)

    centers_out = embs[rep_ids].astype(np.float32)
    return centers_out, rep_ids, np.float32(loss)


def kernel(embs, centers):
    from concourse.bass_utils import run_bass_kernel_spmd

    embs = np.asarray(embs, dtype=np.float32)
    centers = np.asarray(centers, dtype=np.float32)

    nc = _get_program()
    in_maps = make_in_maps(embs, centers)
    res = run_bass_kernel_spmd(nc, in_maps, core_ids=list(range(NCORES)))
    return postprocess(res.results, embs, centers)
